# revision 13
# baseline (speedup 1.0000x reference)
"""D-CLEM forward Trainium2 kernel (nn_D_CLEM_60473139528288).

Sharding: 8 cores = 4 samples x 2 row-halves (32 rows each).
Data path fp16 on-chip (fp32 PSUM accumulation, fp32 residual/output).

Deformable conv strategy:
  - offsets from a 3x3 conv (PE matmuls, shift decomposition)
  - per (tap, pixel) bilinear sample = 2 GPSIMD ap_gathers of fp16
    horizontal PAIRS packed as fp32 (rows y0 and y0+1 share one idx list,
    the second gather uses a +68-element shifted view)
  - blend weights (w00,w01 | w10,w11) applied on DVE with weight planes
    replicated across partitions via a DRAM broadcast read
  - the 4-corner sum is absorbed into the deform matmuls (4 accumulating
    matmuls per tap with stride-2 rhs views)
All activation-grid tensors live on a 68-column padded grid; gather source
is a 70x68 zero-padded image; coordinates are clipped to [-1,64] which is
exactly equivalent to torchvision's valid-masked bilinear gather.
"""
import numpy as np

import concourse.bass as bass
import concourse.mybir as mybir
import concourse.tile as tile
from concourse import bacc, library_config
from concourse.bass_utils import run_bass_kernel_spmd

dt = mybir.dt
F32, F16, I16 = dt.float32, dt.float16, dt.int16
AF = mybir.ActivationFunctionType
OP = mybir.AluOpType

# geometry
B, C, H, W, K, G = 4, 256, 64, 64, 9, 4
CH = 2                      # 128-channel chunks
PW, PH = 68, 70             # padded gather grid
NE = PH * PW                # 4760
NR = 36                     # x_dir local rows (2 junk at bottom)
RBR = 4                     # rows per deform block
NRB = 9                     # deform blocks
JT = RBR * PW               # 272 idx per tap per block
JB = K * JT                 # 2448 idx per block
XDN = 34 * PW               # x_dense valid span (rows 0..33)
ON = 32 * PW                # output window (rows 1..32)
OCR = 38                    # xoc rows (offset-conv input)
OCW = 70                    # xoc cols

_NC_CACHE = None


def build_program():
    nc = bacc.Bacc("TRN2", target_bir_lowering=False, debug=False, num_devices=8)

    # ---------------- DRAM I/O ----------------
    xi_in = nc.dram_tensor("xi", [CH, 128, NE], F32, kind="ExternalInput")
    xoc_in = nc.dram_tensor("xoc", [CH, 128, OCR, OCW], F16, kind="ExternalInput")
    xp_in = nc.dram_tensor("xp", [CH, 128, 34, PW], F16, kind="ExternalInput")
    xres_in = nc.dram_tensor("xres", [CH, 128, 32, 64], F32, kind="ExternalInput")
    rowp_in = nc.dram_tensor("rowp", [81, JT], F32, kind="ExternalInput")
    colp_in = nc.dram_tensor("colp", [81, JT], F32, kind="ExternalInput")
    mask_in = nc.dram_tensor("mask", [128, 2], F32, kind="ExternalInput")
    wofft_in = nc.dram_tensor("wofft", [K, CH, 128, 18], F16, kind="ExternalInput")
    bofft_in = nc.dram_tensor("bofft", [18, 1], F32, kind="ExternalInput")
    wdeft_in = nc.dram_tensor("wdeft", [K, CH, 128, 128], F16, kind="ExternalInput")
    wxt_in = nc.dram_tensor("wxt", [4, CH, 128, 128], F16, kind="ExternalInput")
    wg1t_in = nc.dram_tensor("wg1t", [K, CH, 128, 64], F16, kind="ExternalInput")
    sa_in = nc.dram_tensor("sa", [64, 1], F32, kind="ExternalInput")
    ba_in = nc.dram_tensor("ba", [64, 1], F32, kind="ExternalInput")
    wg2t_in = nc.dram_tensor("wg2t", [CH, 64, 128], F16, kind="ExternalInput")
    bg2_in = nc.dram_tensor("bg2", [128, CH], F32, kind="ExternalInput")
    wott_in = nc.dram_tensor("wott", [CH, CH, 128, 128], F16, kind="ExternalInput")
    so_in = nc.dram_tensor("so", [128, CH], F32, kind="ExternalInput")
    bo_in = nc.dram_tensor("bo", [128, CH], F32, kind="ExternalInput")
    out_dram = nc.dram_tensor("out", [CH, 128, 32, 64], F32, kind="ExternalOutput")

    # internal DRAM scratch
    off_dram = nc.dram_tensor("off_scr", [18, NR * PW], F32, kind="Internal")
    idx_dram = nc.dram_tensor("idx_scr", [81, JT], I16, kind="Internal")
    w_dram = nc.dram_tensor("w_scr", [NRB, 2, JB, 2], F16, kind="Internal")

    with tile.TileContext(nc) as tc:
        nc.gpsimd.load_library(library_config.ap_gather)

        import contextlib
        stack = contextlib.ExitStack()
        cpool = stack.enter_context(tc.tile_pool(name="const", bufs=1))
        mpool = stack.enter_context(tc.tile_pool(name="main", bufs=1))
        ppool_big = stack.enter_context(tc.tile_pool(name="psbig", bufs=2, space="PSUM"))

        # ---------------- constant/persistent loads ----------------
        wofft = cpool.tile([128, K, CH, 18], F16, name="wofft_t")
        nc.sync.dma_start(wofft[:], wofft_in[:].rearrange("k c p o -> p k c o"))
        wdeft = cpool.tile([128, K, CH, 128], F16, name="wdeft_t")
        nc.sync.dma_start(wdeft[:], wdeft_in[:].rearrange("k c p o -> p k c o"))
        wxt = cpool.tile([128, 4, CH, 128], F16, name="wxt_t")
        nc.sync.dma_start(wxt[:], wxt_in[:].rearrange("k c p o -> p k c o"))
        wg1t = cpool.tile([128, K, CH, 64], F16, name="wg1t_t")
        nc.sync.dma_start(wg1t[:], wg1t_in[:].rearrange("k c p o -> p k c o"))
        wg2t = cpool.tile([64, CH, 128], F16, name="wg2t_t")
        nc.sync.dma_start(wg2t[:], wg2t_in[:].rearrange("c p o -> p c o"))
        wott = cpool.tile([128, CH, CH, 128], F16, name="wott_t")
        nc.sync.dma_start(wott[:], wott_in[:].rearrange("k c p o -> p k c o"))
        bofft = cpool.tile([18, 1], F32)
        nc.sync.dma_start(bofft[:], bofft_in[:])
        sa = cpool.tile([64, 1], F32)
        nc.sync.dma_start(sa[:], sa_in[:])
        ba = cpool.tile([64, 1], F32)
        nc.sync.dma_start(ba[:], ba_in[:])
        bg2 = cpool.tile([128, CH], F32)
        nc.sync.dma_start(bg2[:], bg2_in[:])
        so = cpool.tile([128, CH], F32)
        nc.sync.dma_start(so[:], so_in[:])
        bo = cpool.tile([128, CH], F32)
        nc.sync.dma_start(bo[:], bo_in[:])
        maskt = cpool.tile([128, 2], F32)
        nc.sync.dma_start(maskt[:], mask_in[:])

        xi = mpool.tile([128, CH, NE], F32)
        for ch in range(CH):
            nc.sync.dma_start(xi[:, ch, :], xi_in[ch])
        xp16 = mpool.tile([128, CH, 34 * PW], F16)
        for ch in range(CH):
            nc.sync.dma_start(xp16[:, ch, :], xp_in[ch].rearrange("p r c -> p (r c)"))

        xdir = mpool.tile([128, CH, NR * PW], F16)
        xdense = mpool.tile([128, CH, NR * PW + 2], F16)
        a16 = mpool.tile([64, ON], F16)
        attn = mpool.tile([128, CH, ON], F16)
        xa16 = mpool.tile([128, CH, ON], F16)

        # ================= S1: offset conv =================
        with tc.tile_pool(name="early", bufs=1) as epool, \
             tc.tile_pool(name="psoff", bufs=2, space="PSUM") as po_off:
            xoc = epool.tile([128, CH, OCR, OCW], F16)
            for ch in range(CH):
                nc.sync.dma_start(xoc[:, ch, :, :], xoc_in[ch])
            offs = epool.tile([18, NR * PW], F32)

            row_chunks = [(0, 7), (7, 7), (14, 7), (21, 7), (28, 7), (35, 1)]
            for (r0, nr) in row_chunks:
                n = nr * PW
                ps = po_off.tile([18, 476], F32, name="psoff")
                first = True
                for k in range(K):
                    di, dj = k // 3, k % 3
                    for ch in range(CH):
                        rhs = xoc[:, ch, di + r0: di + r0 + nr, dj: dj + PW]
                        nc.tensor.matmul(ps[:, :n], wofft[:, k, ch, :], rhs,
                                         start=first, stop=(k == K - 1 and ch == CH - 1))
                        first = False
                nc.scalar.activation(offs[:, r0 * PW:(r0 + nr) * PW], ps[:, :n],
                                     AF.Identity, bias=bofft[:], scale=1.0)
            nc.sync.dma_start(off_dram[:], offs[:])

            # ============ S2/S3: index + weight pipeline ============
            dyt = epool.tile([81, JT], F32)
            dxt = epool.tile([81, JT], F32)
            # dram fancy read: partition (k*9+rb) <- off[2k (+1), rb-block rows]
            # off_dram [18, 36*68]; block rb covers rows 4rb..4rb+3 -> cols rb*272..+272
            offv = off_dram[:].rearrange("c (rb j) -> c rb j", rb=NRB)
            for k in range(K):
                nc.sync.dma_start(dyt[k * NRB:(k + 1) * NRB, :], offv[2 * k])
                nc.sync.dma_start(dxt[k * NRB:(k + 1) * NRB, :], offv[2 * k + 1])

            rowp = epool.tile([81, JT], F32)
            nc.sync.dma_start(rowp[:], rowp_in[:])
            colp = epool.tile([81, JT], F32)
            nc.sync.dma_start(colp[:], colp_in[:])

            MAGIC = 8388608.0  # 2^23: (x+MAGIC)-MAGIC == round-half-even(x)

            def floor_frac(coord, tag):
                t = epool.tile([81, JT], F32, name=f"ff_t_{tag}")
                nc.vector.tensor_scalar(t[:], coord[:], MAGIC, None, OP.add)
                nc.vector.tensor_scalar(t[:], t[:], MAGIC, None, OP.subtract)
                gt = epool.tile([81, JT], F32, name=f"ff_gt_{tag}")
                nc.vector.tensor_tensor(gt[:], t[:], coord[:], OP.is_gt)
                fl = epool.tile([81, JT], F32, name=f"ff_fl_{tag}")
                nc.vector.tensor_tensor(fl[:], t[:], gt[:], OP.subtract)
                fr = epool.tile([81, JT], F32, name=f"ff_fr_{tag}")
                nc.vector.tensor_tensor(fr[:], coord[:], fl[:], OP.subtract)
                return fl, fr

            py1 = epool.tile([81, JT], F32)
            nc.vector.tensor_tensor(py1[:], dyt[:], rowp[:], OP.add)
            nc.vector.tensor_scalar(py1[:], py1[:], 0.0, 65.0, OP.max, OP.min)
            y0, fy = floor_frac(py1, "y")

            px1 = epool.tile([81, JT], F32)
            nc.vector.tensor_tensor(px1[:], dxt[:], colp[:], OP.add)
            nc.vector.tensor_scalar(px1[:], px1[:], 0.0, 65.0, OP.max, OP.min)
            x0, fx = floor_frac(px1, "x")

            idxf = epool.tile([81, JT], F32)
            nc.vector.scalar_tensor_tensor(idxf[:], y0[:], float(PW), x0[:],
                                           OP.mult, OP.add)
            idx16 = epool.tile([81, JT], I16)
            nc.vector.tensor_copy(
                idx16[:].rearrange("q (cr c16) -> q cr c16", c16=17),
                idxf[:].rearrange("q (c16 cr) -> q cr c16", cr=16))
            nc.sync.dma_start(idx_dram[:], idx16[:])

            # blend weights (fp16): w0 = (1-fy)*(1-fx | fx), w1 = fy*(1-fx | fx)
            gy = epool.tile([81, JT], F16)   # 1-fy
            nc.vector.tensor_scalar(gy[:], fy[:], -1.0, 1.0, OP.mult, OP.add)
            gx = epool.tile([81, JT], F16)   # 1-fx
            nc.vector.tensor_scalar(gx[:], fx[:], -1.0, 1.0, OP.mult, OP.add)
            hy = epool.tile([81, JT], F16)
            nc.vector.tensor_copy(hy[:], fy[:])
            hx = epool.tile([81, JT], F16)
            nc.vector.tensor_copy(hx[:], fx[:])
            w00 = epool.tile([81, JT], F16)
            nc.vector.tensor_tensor(w00[:], gy[:], gx[:], OP.mult)
            w01 = epool.tile([81, JT], F16)
            nc.vector.tensor_tensor(w01[:], gy[:], hx[:], OP.mult)
            w10 = epool.tile([81, JT], F16)
            nc.vector.tensor_tensor(w10[:], hy[:], gx[:], OP.mult)
            w11 = epool.tile([81, JT], F16)
            nc.vector.tensor_tensor(w11[:], hy[:], hx[:], OP.mult)

            # store interleaved pair planes to DRAM: w_dram[rb, r, (k j), s]
            wv = w_dram[:].rearrange("rb r (k j) s -> k rb r j s", k=K)
            for k in range(K):
                nc.sync.dma_start(wv[k, :, 0, :, 0], w00[k * NRB:(k + 1) * NRB, :])
                nc.sync.dma_start(wv[k, :, 0, :, 1], w01[k * NRB:(k + 1) * NRB, :])
                nc.sync.dma_start(wv[k, :, 1, :, 0], w10[k * NRB:(k + 1) * NRB, :])
                nc.sync.dma_start(wv[k, :, 1, :, 1], w11[k * NRB:(k + 1) * NRB, :])

        # ================= S5-S10: deform gather + matmul =================
        with tc.tile_pool(name="gidx", bufs=2) as gip, \
             tc.tile_pool(name="gw", bufs=2) as gwp, \
             tc.tile_pool(name="gg", bufs=2) as ggp, \
             tc.tile_pool(name="psxd", bufs=4, space="PSUM") as po_xd:
            for rb in range(NRB):
                idxw = gip.tile([128, JB // 16], I16, name="idxw")
                # idx_dram free pos c' = cr*17 + c16 holds idx of flat pos c16*16+cr;
                # wrapped tile[p, 17k+c16] = idx_{j=16*(17k+c16)+p} -> src (p,k,c16)
                srcv = idx_dram[:].rearrange(
                    "(k rb) (p c16) -> rb p k c16", rb=NRB, c16=17)[rb]
                for g in range(8):
                    dst = idxw[16 * g:16 * (g + 1), :].rearrange(
                        "p (k c16) -> p k c16", k=K)
                    nc.sync.dma_start(dst, srcv)
                w0rep = gwp.tile([128, JB * 2], F16, name="w0rep")
                nc.sync.dma_start(w0rep[:], w_dram[rb:rb + 1, 0].rearrange(
                    "one j s -> one (j s)").to_broadcast([128, JB * 2]))
                w1rep = gwp.tile([128, JB * 2], F16, name="w1rep")
                nc.sync.dma_start(w1rep[:], w_dram[rb:rb + 1, 1].rearrange(
                    "one j s -> one (j s)").to_broadcast([128, JB * 2]))

                for ch in range(CH):
                    g0 = ggp.tile([128, JB], F32, name="g0")
                    nc.gpsimd.ap_gather(g0[:], xi[:, ch, :], idxw[:],
                                        channels=128, num_elems=NE, d=1, num_idxs=JB)
                    g1 = ggp.tile([128, JB], F32, name="g1")
                    nc.gpsimd.ap_gather(g1[:], xi[:, ch, PW:], idxw[:],
                                        channels=128, num_elems=NE - PW, d=1, num_idxs=JB)
                    g0h = g0[:].bitcast(F16)
                    g1h = g1[:].bitcast(F16)
                    nc.vector.tensor_tensor(g0h, g0h, w0rep[:], OP.mult)
                    nc.vector.tensor_tensor(g1h, g1h, w1rep[:], OP.mult)

                    ps = po_xd.tile([128, JT], F32, name="psxd")
                    first = True
                    for k in range(K):
                        for gh in (g0h, g1h):
                            pv = gh.rearrange("p (j s) -> p j s", s=2)
                            for s in range(2):
                                rhs = pv[:, k * JT:(k + 1) * JT, s]
                                nc.tensor.matmul(
                                    ps[:], wdeft[:, k, ch, :], rhs,
                                    start=first,
                                    stop=(k == K - 1 and gh is g1h and s == 1))
                                first = False
                    nc.scalar.copy(xdir[:, ch, rb * JT:(rb + 1) * JT], ps[:])

        # ================= S11: cross conv -> x_dense =================
        chunks2312 = [(0, 512), (512, 512), (1024, 512), (1536, 512), (2048, 264)]
        for oc in range(CH):
            for (s0, n) in chunks2312:
                ps = ppool_big.tile([128, 512], F32, name="psbig")
                first = True
                for ch in range(CH):
                    nc.tensor.matmul(ps[:, :n], wxt[:, ch, oc, :],
                                     xdir[:, ch, s0:s0 + n], start=first, stop=False)
                    first = False
                for ch in range(CH):
                    nc.tensor.matmul(ps[:, :n], wxt[:, 2 + ch, oc, :],
                                     xp16[:, ch, s0:s0 + n], start=False,
                                     stop=(ch == CH - 1))
                nc.scalar.copy(xdense[:, oc, 1 + s0:1 + s0 + n], ps[:, :n])

        # masking: lead/tail, pad cols, boundary rows
        for oc in range(CH):
            nc.vector.memset(xdense[:, oc, 0:1], 0.0)
            nc.vector.memset(xdense[:, oc, 1 + XDN:NR * PW + 2], 0.0)
            xdv = xdense[:, oc, 1:1 + XDN].rearrange("p (r c) -> p r c", c=PW)
            nc.vector.memset(xdv[:, :, 0:1], 0.0)
            nc.vector.memset(xdv[:, :, 65:68], 0.0)
            nc.vector.tensor_scalar_mul(xdv[:, 0, :], xdv[:, 0, :], maskt[:, 0:1])
            nc.vector.tensor_scalar_mul(xdv[:, 33, :], xdv[:, 33, :], maskt[:, 1:2])

        # ================= S12: g1 conv + bn + silu =================
        chunks2176 = [(0, 512), (512, 512), (1024, 512), (1536, 512), (2048, 128)]
        tsig = mpool.tile([64, ON], F16)
        tz = mpool.tile([64, ON], F16)
        for (s0, n) in chunks2176:
            ps = ppool_big.tile([128, 512], F32, name="psbig")
            first = True
            for k in range(K):
                di, dj = k // 3, k % 3
                base = di * PW + dj
                for ch in range(CH):
                    nc.tensor.matmul(ps[:64, :n], wg1t[:, k, ch, :],
                                     xdense[:, ch, base + s0: base + s0 + n],
                                     start=first, stop=(k == K - 1 and ch == CH - 1))
                    first = False
            nc.scalar.activation(tsig[:, s0:s0 + n], ps[:64, :n], AF.Sigmoid,
                                 bias=ba[:], scale=sa[:])
            nc.scalar.activation(tz[:, s0:s0 + n], ps[:64, :n], AF.Identity,
                                 bias=ba[:], scale=sa[:])
        nc.vector.tensor_tensor(a16[:], tsig[:], tz[:], OP.mult)

        # ================= S13: g2 conv -> attn =================
        for oc in range(CH):
            for (s0, n) in chunks2176:
                ps = ppool_big.tile([128, 512], F32, name="psbig")
                nc.tensor.matmul(ps[:, :n], wg2t[:, oc, :], a16[:, s0:s0 + n],
                                 start=True, stop=True)
                nc.scalar.activation(attn[:, oc, s0:s0 + n], ps[:, :n], AF.Sigmoid,
                                     bias=bg2[:, oc:oc + 1], scale=1.0)

        # ================= S14: xa = x_dense * attn =================
        for ch in range(CH):
            nc.vector.tensor_tensor(xa16[:, ch, :], xdense[:, ch, 1 + PW:1 + PW + ON],
                                    attn[:, ch, :], OP.mult)

        # ================= S15/S16: out conv + bn + silu + residual ========
        with tc.tile_pool(name="late", bufs=1) as lpool:
            xrest = lpool.tile([128, CH, ON], F32)
            nc.vector.memset(xrest[:], 0.0)
            for ch in range(CH):
                dstv = xrest[:, ch, :].rearrange("p (r c) -> p r c", c=PW)
                nc.sync.dma_start(dstv[:, :, 1:65], xres_in[ch])
            outt = lpool.tile([128, CH, ON], F32)
            tso = lpool.tile([128, ON], F32, name="tso")
            tzo = lpool.tile([128, ON], F32, name="tzo")
            for oc in range(CH):
                for (s0, n) in chunks2176:
                    ps = ppool_big.tile([128, 512], F32, name="psbig")
                    for ch in range(CH):
                        nc.tensor.matmul(ps[:, :n], wott[:, ch, oc, :],
                                         xa16[:, ch, s0:s0 + n],
                                         start=(ch == 0), stop=(ch == CH - 1))
                    nc.scalar.activation(tso[:, s0:s0 + n], ps[:, :n], AF.Sigmoid,
                                         bias=bo[:, oc:oc + 1], scale=so[:, oc:oc + 1])
                    nc.scalar.activation(tzo[:, s0:s0 + n], ps[:, :n], AF.Identity,
                                         bias=bo[:, oc:oc + 1], scale=so[:, oc:oc + 1])
                nc.vector.tensor_tensor(tso[:], tso[:], tzo[:], OP.mult)
                nc.vector.tensor_tensor(outt[:, oc, :], tso[:], xrest[:, oc, :], OP.add)
                srcv = outt[:, oc, :].rearrange("p (r c) -> p r c", c=PW)
                nc.sync.dma_start(out_dram[oc], srcv[:, :, 1:65])

        stack.close()

    nc.compile()
    return nc


# ======================= host side =======================

def _f16(a):
    return np.asarray(a, dtype=np.float16)


def prep_inputs(inputs):
    """Build the 8 per-core input maps."""
    x = np.asarray(inputs["x"], np.float32)
    x_prev = np.asarray(inputs["x_prev"], np.float32)
    w_off = np.asarray(inputs["w_off"], np.float32)
    b_off = np.asarray(inputs["b_off"], np.float32)
    w_def = np.asarray(inputs["w_def"], np.float32)
    w_cross = np.asarray(inputs["w_cross"], np.float32)
    w_g1 = np.asarray(inputs["w_g1"], np.float32)
    b_g1 = np.asarray(inputs["b_g1"], np.float32)
    g1_gamma = np.asarray(inputs["g1_gamma"], np.float32)
    g1_beta = np.asarray(inputs["g1_beta"], np.float32)
    g1_mean = np.asarray(inputs["g1_mean"], np.float32)
    g1_var = np.asarray(inputs["g1_var"], np.float32)
    w_g2 = np.asarray(inputs["w_g2"], np.float32)
    b_g2 = np.asarray(inputs["b_g2"], np.float32)
    w_out = np.asarray(inputs["w_out"], np.float32)
    b_out = np.asarray(inputs["b_out"], np.float32)
    o_gamma = np.asarray(inputs["o_gamma"], np.float32)
    o_beta = np.asarray(inputs["o_beta"], np.float32)
    o_mean = np.asarray(inputs["o_mean"], np.float32)
    o_var = np.asarray(inputs["o_var"], np.float32)

    eps = 1e-5
    inv_a = g1_gamma / np.sqrt(g1_var + eps)
    bias_a = b_g1 * inv_a + (g1_beta - g1_mean * inv_a)
    inv_o = o_gamma / np.sqrt(o_var + eps)
    bias_o = b_out * inv_o + (o_beta - o_mean * inv_o)

    # gather image [B, C, 70, 68] rows g-1..68 cols g-1..66
    ximg = np.zeros((B, C, PH, PW), np.float16)
    ximg[:, :, 1:65, 1:65] = _f16(x)
    flat = ximg.reshape(B, C, NE)
    xi_pairs = np.zeros((B, C, NE, 2), np.float16)
    xi_pairs[..., 0] = flat
    xi_pairs[:, :, :-1, 1] = flat[..., 1:]
    xi_packed = np.ascontiguousarray(xi_pairs).view(np.float32)[..., 0]  # [B,C,NE]

    # offset-conv input [B, C, 72, 70]: rows g-2..69, cols g-2..67
    xocimg = np.zeros((B, C, 72, OCW), np.float16)
    xocimg[:, :, 2:66, 2:66] = _f16(x)

    # x_prev padded [B, C, 66, 68]: rows g-1..64, cols g-1..66
    xpimg = np.zeros((B, C, 66, PW), np.float16)
    xpimg[:, :, 1:65, 1:65] = _f16(x_prev)

    ki = np.arange(K) // 3 - 1
    kj = np.arange(K) % 3 - 1
    r4 = np.arange(RBR)[:, None]
    cc = np.arange(PW)[None, :]

    # weights (shared across cores)
    wofft = np.zeros((K, CH, 128, 18), np.float16)
    wdeft = np.zeros((K, CH, 128, 128), np.float16)
    wg1t = np.zeros((K, CH, 128, 64), np.float16)
    for k in range(K):
        di, dj = k // 3, k % 3
        for ch in range(CH):
            wofft[k, ch] = _f16(w_off[:, ch * 128:(ch + 1) * 128, di, dj].T)
            wg1t[k, ch] = _f16(w_g1[:, ch * 128:(ch + 1) * 128, di, dj].T)
            for a in range(2):
                g = 2 * ch + a
                blk = _f16(w_def[g * 64:(g + 1) * 64, :, di, dj].T)  # [64c, 64o]
                wdeft[k, ch, 64 * a:64 * (a + 1), 64 * a:64 * (a + 1)] = blk
    wxt = np.zeros((4, CH, 128, 128), np.float16)
    for cin in range(4):
        for oc in range(CH):
            wxt[cin, oc] = _f16(
                w_cross[oc * 128:(oc + 1) * 128, cin * 128:(cin + 1) * 128, 0, 0].T)
    wg2t = np.zeros((CH, 64, 128), np.float16)
    for oc in range(CH):
        wg2t[oc] = _f16(w_g2[oc * 128:(oc + 1) * 128, :, 0, 0].T)
    wott = np.zeros((CH, CH, 128, 128), np.float16)
    for cin in range(CH):
        for oc in range(CH):
            wott[cin, oc] = _f16(
                w_out[oc * 128:(oc + 1) * 128, cin * 128:(cin + 1) * 128, 0, 0].T)

    colp = np.zeros((K, NRB, RBR, PW), np.float32)
    for k in range(K):
        colp[k] = (cc + kj[k]).astype(np.float32)

    shared = {
        "colp": colp.reshape(81, JT),
        "wofft": wofft, "bofft": b_off.reshape(18, 1),
        "wdeft": wdeft, "wxt": wxt, "wg1t": wg1t,
        "sa": inv_a.reshape(64, 1), "ba": bias_a.reshape(64, 1),
        "wg2t": wg2t,
        "bg2": b_g2.reshape(CH, 128).T.astype(np.float32).copy(),
        "wott": wott,
        "so": inv_o.reshape(CH, 128).T.astype(np.float32).copy(),
        "bo": bias_o.reshape(CH, 128).T.astype(np.float32).copy(),
    }

    in_maps = []
    for core in range(8):
        b, half = core // 2, core % 2
        h0 = half * 32
        rowp = np.zeros((K, NRB, RBR, PW), np.float32)
        for k in range(K):
            for rb in range(NRB):
                rowp[k, rb] = h0 + rb * RBR + r4 + ki[k]
        m = {
            "xi": xi_packed[b].reshape(CH, 128, NE),
            "xoc": xocimg[b, :, h0:h0 + OCR, :].reshape(CH, 128, OCR, OCW),
            "xp": xpimg[b, :, h0:h0 + 34, :].reshape(CH, 128, 34, PW),
            "xres": x[b, :, h0:h0 + 32, :].reshape(CH, 128, 32, 64),
            "rowp": rowp.reshape(81, JT),
            "colp": shared["colp"],
            "mask": np.broadcast_to(
                np.array([1.0 if h0 > 0 else 0.0,
                          1.0 if h0 + 32 < 64 else 0.0], np.float32),
                (128, 2)).copy(),
        }
        m.update({k: v for k, v in shared.items() if k != "colp"})
        in_maps.append(m)
    return in_maps


def assemble_output(results):
    out = np.zeros((B, C, H, W), np.float32)
    for core, r in enumerate(results):
        b, half = core // 2, core % 2
        h0 = half * 32
        out[b, :, h0:h0 + 32, :] = r["out"].reshape(C, 32, 64)
    return out


def kernel(**inputs):
    global _NC_CACHE
    if _NC_CACHE is None:
        _NC_CACHE = build_program()
    in_maps = prep_inputs(inputs)
    res = run_bass_kernel_spmd(_NC_CACHE, in_maps, core_ids=list(range(8)))
    return assemble_output(res.results)
